# revision 1
# baseline (speedup 1.0000x reference)
"""Causal GQA attention (qk-norm + rope) on 8 TRN2 NeuronCores.

Sharding: tensor-parallel over heads. Core c owns Q heads {2c, 2c+1} and
KV group c//2 (w_qkv column-parallel, w_o row-parallel). Each core
computes a full-shape partial of the output projection; the host sums
the 8 partials (row-parallel w_o => partial sums, no on-device
collective).

Per-core pipeline (all matmuls bf16 on PE, fp32 PSUM accumulate):
  1. x^T loaded straight from DRAM via DMA xbar transpose (bf16).
  2. qkv = x @ w_qkv_c in natural [s, c] layout, processed in 512-row
     super-blocks: L2 qk-norm (free-dim reduce) + rope batched across
     4 blocks x 3 heads per DVE op, then PE transpose of q-hat/k-hat
     into [hd, s] for attention.
  3. Flash-style causal attention per head: S^T[k, q] blocks on PE,
     exp on ACT (scale 1/8 folded in; scores are bounded by +-1/8 after
     qk-norm so no max subtraction), causal mask applied post-exp as a
     0/1 bf16 multiply, A^T V accumulation on PE with an appended ones
     column producing the softmax denominator for free.
  4. y_partial = out_heads @ w_o_rows, DMA'd out per tile.
"""

import os

import numpy as np
import ml_dtypes

import concourse.bass as bass
import concourse.tile as tile
from concourse import bacc, mybir
from concourse.bass_utils import run_bass_kernel_spmd

F32 = mybir.dt.float32
BF16 = mybir.dt.bfloat16
AF = mybir.ActivationFunctionType
OP = mybir.AluOpType

T = 4096          # sequence length
D = 1024          # d_model
HD = 64           # head dim
NB = T // 128     # 32 seq blocks of 128
NSB = T // 512    # 8 super blocks of 512
NCORES = 8
THETA = 10000.0

_built = {}


class _nullctx:
    def __enter__(self):
        return None

    def __exit__(self, *a):
        return False


def _emit(tc, nc, xb_d, wqkv_d, wo_d, cos_d, sin_d, mask_d, id_d, ones_d, y_d):
    with (
        tc.tile_pool(name="pers", bufs=1) as pers,
        tc.tile_pool(name="stage", bufs=2) as stage,
    ):
        # persistent SBUF tensors
        xT = pers.tile([128, 8, T], BF16)       # x^T, d-chunk j on partitions
        QT0 = pers.tile([64, T], BF16)          # q-hat^T head 0
        QT1 = pers.tile([64, T], BF16)          # q-hat^T head 1
        KT = pers.tile([64, T], BF16)           # k-hat^T
        VT = pers.tile([128, NB, 65], BF16)     # per k-block [V | 1]
        OT = pers.tile([128, T], BF16)          # normalized attn out^T (2 heads)
        wqkv_b = pers.tile([128, 8, 256], BF16)
        wo_b = pers.tile([128, D], BF16)
        cos_sb = pers.tile([128, NSB, 12, 32], F32)
        sin_sb = pers.tile([128, NSB, 12, 32], F32)
        mask_sb = pers.tile([128, 4, 512], BF16)
        id_sb = pers.tile([128, 128], BF16)
        ones_sb = pers.tile([1, 64], BF16)

        wqkv_f = stage.tile([128, 8, 256], F32, tag="wq_f")
        wo_f = stage.tile([128, D], F32, tag="wo_f")
        nc.sync.dma_start(wqkv_f[:], wqkv_d.rearrange("(j p) c -> p j c", p=128))
        nc.sync.dma_start(wo_f[:], wo_d[:])
        nc.vector.tensor_copy(wqkv_b[:], wqkv_f[:])
        nc.vector.tensor_copy(wo_b[:], wo_f[:])
        nc.sync.dma_start(cos_sb[:], cos_d[:])
        nc.sync.dma_start(sin_sb[:], sin_d[:])
        nc.sync.dma_start(mask_sb[:], mask_d.rearrange("i p q -> p i q"))
        nc.sync.dma_start(id_sb[:], id_d[:])
        nc.sync.dma_start(ones_sb[:], ones_d[:])
        nc.vector.memset(VT[:, :, 64], 1.0)

        # ---- phase 1: qkv projection + qk-norm + rope, per 512-row superblock
        with (
            tc.tile_pool(name="p1w", bufs=3) as p1w,
            tc.tile_pool(name="p1ps", bufs=2, space="PSUM") as p1ps,
            tc.tile_pool(name="p1pq", bufs=2, space="PSUM") as p1pq,
            tc.tile_pool(name="p1pk", bufs=2, space="PSUM") as p1pk,
            (tc.For_i(0, int(os.environ.get("K_REP1", "1")), 1,
                      hint_engines=(mybir.EngineType.PE,
                                    mybir.EngineType.Activation,
                                    mybir.EngineType.DVE,
                                    mybir.EngineType.SP))
             if os.environ.get("K_REP1", "1") != "1" else _nullctx()),
        ):
            for S in range(NSB):
                # x^T slice for this superblock via DMA xbar transpose
                for j in range(8):
                    nc.sync.dma_start(
                        xT[:, j, S * 512:(S + 1) * 512],
                        xb_d[S * 512:(S + 1) * 512, 128 * j:128 * (j + 1)],
                        transpose=True)
                qkvp = p1ps.tile([128, 4, 256], F32, tag="qkvp")
                for b in range(4):
                    sb = 4 * S + b
                    for j in range(8):
                        nc.tensor.matmul(qkvp[:, b, :],
                                         xT[:, j, sb * 128:(sb + 1) * 128],
                                         wqkv_b[:, j, :],
                                         start=(j == 0), stop=(j == 7))

                # v slice straight to VT (no norm/rope)
                nc.vector.tensor_copy(VT[:, 4 * S:4 * S + 4, 0:64],
                                      qkvp[:, :, 192:256])

                qk_s = p1w.tile([128, 4, 192], F32, tag="qk_s")
                nc.scalar.copy(qk_s[:], qkvp[:, :, 0:192])

                sq = p1w.tile([128, 4, 192], F32, tag="sq")
                ss = p1w.tile([128, 4, 3], F32, tag="ss")
                nc.scalar.square(sq[:], qk_s[:])
                nc.vector.reduce_sum(ss[:], sq.rearrange("p b (h d) -> p b h d", h=3),
                                     axis=mybir.AxisListType.X)
                srt = p1w.tile([128, 4, 3], F32, tag="srt")
                nc.scalar.sqrt(srt[:], ss[:])
                invn = p1w.tile([128, 4, 3], F32, tag="invn")
                nc.vector.reciprocal(invn[:], srt[:])

                # batched rotate-half rope over [128, 4 blocks, 3 heads, 32]
                qv = qk_s.rearrange("p b (h d) -> p b h d", h=3)
                t1, t2 = qv[:, :, :, 0:32], qv[:, :, :, 32:64]
                cs = cos_sb[:, S].rearrange("p (b h) c -> p b h c", b=4)
                sn = sin_sb[:, S].rearrange("p (b h) c -> p b h c", b=4)
                r1 = p1w.tile([128, 4, 3, 32], F32, tag="r1")
                r2 = p1w.tile([128, 4, 3, 32], F32, tag="r2")
                rot = p1w.tile([128, 4, 3, 64], F32, tag="rot")
                nc.vector.tensor_mul(r1[:], t1, cs)
                nc.vector.tensor_mul(r2[:], t2, sn)
                nc.vector.tensor_sub(rot[:, :, :, 0:32], r1[:], r2[:])
                nc.vector.tensor_mul(r1[:], t2, cs)
                nc.vector.tensor_mul(r2[:], t1, sn)
                nc.vector.tensor_add(rot[:, :, :, 32:64], r1[:], r2[:])

                # normalize (scale by 1/||.||) and cast to bf16
                qhat = p1w.tile([128, 4, 192], BF16, tag="qhat")
                qh = qhat.rearrange("p b (h d) -> p b h d", h=3)
                for b in range(4):
                    for h3 in range(3):
                        nc.vector.tensor_scalar_mul(
                            qh[:, b, h3], rot[:, b, h3],
                            invn[:, b, h3:h3 + 1])

                # transpose q-hat / k-hat into [hd, s] layout
                pq = p1pq.tile([128, 4, 128], BF16, tag="pq")
                pk = p1pk.tile([64, 4, 128], BF16, tag="pk")
                for b in range(4):
                    nc.tensor.transpose(pq[:, b, :], qhat[:, b, 0:128], id_sb[:])
                    nc.tensor.transpose(pk[:, b, :], qhat[:, b, 128:192], id_sb[:])
                s0 = S * 512
                nc.scalar.copy(QT0[:, s0:s0 + 512],
                               pq[0:64].rearrange("p b s -> p (b s)"))
                nc.scalar.copy(QT1[:, s0:s0 + 512],
                               pq[64:128].rearrange("p b s -> p (b s)"))
                nc.scalar.copy(KT[:, s0:s0 + 512],
                               pk.rearrange("p b s -> p (b s)"))

        # ---- phase 2: causal attention per head + output projection
        with (
            tc.tile_pool(name="p2s", bufs=2, space="PSUM") as p2s,
            tc.tile_pool(name="p2av", bufs=2, space="PSUM") as p2av,
            tc.tile_pool(name="p2y", bufs=2, space="PSUM") as p2y,
            tc.tile_pool(name="p2sb", bufs=6) as p2sb,
            tc.tile_pool(name="p2n", bufs=2) as p2n,
            (tc.For_i(0, int(os.environ.get("K_REP", "1")), 1,
                      hint_engines=(mybir.EngineType.PE,
                                    mybir.EngineType.Activation,
                                    mybir.EngineType.DVE,
                                    mybir.EngineType.SP))
             if os.environ.get("K_REP", "1") != "1" else _nullctx()),
        ):
            for qc in range(8):          # 512-wide q chunks
                q0 = qc * 512
                for h in range(2):
                    qth = QT0 if h == 0 else QT1
                    av = p2av.tile([65, 512], F32, tag="av")
                    for p in range(2 * qc):        # k-block pairs below diagonal
                        sp = p2s.tile([128, 2, 512], F32, tag="sp")
                        for j in range(2):
                            kb = 2 * p + j
                            nc.tensor.matmul(sp[:, j, :],
                                             KT[:, kb * 128:(kb + 1) * 128],
                                             qth[:, q0:q0 + 512],
                                             start=True, stop=True)
                        ap = p2sb.tile([128, 2, 512], BF16, tag="ap")
                        nc.scalar.activation(ap[:], sp[:], AF.Exp, scale=0.125)
                        for j in range(2):
                            kb = 2 * p + j
                            nc.tensor.matmul(av[:], VT[:, kb, :], ap[:, j, :],
                                             start=(kb == 0), stop=False,
                                             skip_group_check=True)
                    for p in range(2):             # diagonal window (4 blocks)
                        sp = p2s.tile([128, 2, 512], F32, tag="sp")
                        for j in range(2):
                            kb = 4 * qc + 2 * p + j
                            nc.tensor.matmul(sp[:, j, :],
                                             KT[:, kb * 128:(kb + 1) * 128],
                                             qth[:, q0:q0 + 512],
                                             start=True, stop=True)
                        ap = p2sb.tile([128, 2, 512], BF16, tag="ap")
                        nc.scalar.activation(ap[:], sp[:], AF.Exp, scale=0.125)
                        for j in range(2):
                            i = 2 * p + j
                            kb = 4 * qc + i
                            wm = 128 * (i + 1)   # mask is all-ones past col wm
                            nc.vector.tensor_mul(ap[:, j, 0:wm], ap[:, j, 0:wm],
                                                 mask_sb[:, i, 0:wm])
                            nc.tensor.matmul(av[:], VT[:, kb, :], ap[:, j, :],
                                             start=(qc == 0 and i == 0),
                                             stop=(i == 3),
                                             skip_group_check=True)
                    # normalize: row 64 of av is the softmax denominator
                    rec = p2n.tile([1, 512], F32, tag="rec")
                    nc.vector.reciprocal(rec[:], av[64:65, :])
                    bcs = p2n.tile([64, 512], F32, tag="bcs")
                    nc.gpsimd.partition_broadcast(bcs[:], rec[:])
                    nc.vector.tensor_mul(OT[64 * h:64 * h + 64, q0:q0 + 512],
                                         av[0:64, :], bcs[:])
                # output projection for this q chunk (both heads ready)
                for qb in range(4):
                    ot_blk = OT[:, q0 + qb * 128:q0 + (qb + 1) * 128]
                    for nh in range(2):
                        yp = p2y.tile([128, 512], F32, tag="yp")
                        nc.tensor.matmul(yp[:], ot_blk,
                                         wo_b[:, nh * 512:(nh + 1) * 512],
                                         start=True, stop=True)
                        ys = p2sb.tile([128, 512], F32, tag="ys")
                        nc.vector.tensor_copy(ys[:], yp[:])
                        nc.sync.dma_start(
                            y_d[q0 + qb * 128:q0 + (qb + 1) * 128,
                                nh * 512:(nh + 1) * 512], ys[:])


def _build():
    key = (os.environ.get("K_REP", "1"), os.environ.get("K_REP1", "1"))
    if key in _built:
        return _built[key]
    nc = bacc.Bacc("TRN2", target_bir_lowering=False, debug=False)
    xb_d = nc.dram_tensor("xb", [T, D], BF16, kind="ExternalInput").ap()
    wqkv_d = nc.dram_tensor("wqkv", [D, 256], F32, kind="ExternalInput").ap()
    wo_d = nc.dram_tensor("wo", [128, D], F32, kind="ExternalInput").ap()
    cos_d = nc.dram_tensor("cos12", [128, NSB, 12, 32], F32,
                           kind="ExternalInput").ap()
    sin_d = nc.dram_tensor("sin12", [128, NSB, 12, 32], F32,
                           kind="ExternalInput").ap()
    mask_d = nc.dram_tensor("mask", [4, 128, 512], BF16, kind="ExternalInput").ap()
    id_d = nc.dram_tensor("ident", [128, 128], BF16, kind="ExternalInput").ap()
    ones_d = nc.dram_tensor("ones64", [1, 64], BF16, kind="ExternalInput").ap()
    y_d = nc.dram_tensor("y", [T, D], F32, kind="ExternalOutput").ap()
    with tile.TileContext(nc) as tc:
        _emit(tc, nc, xb_d, wqkv_d, wo_d, cos_d, sin_d, mask_d, id_d, ones_d, y_d)
    nc.compile()
    _built[key] = nc
    return nc


def host_inputs(x, w_qkv, w_o):
    """Per-core input dicts (shards + constant tables)."""
    x2 = np.ascontiguousarray(np.asarray(x, np.float32).reshape(T, D))
    xb = x2.astype(ml_dtypes.bfloat16)
    w_qkv = np.asarray(w_qkv, np.float32)
    w_o = np.asarray(w_o, np.float32)

    half = HD // 2
    inv_freq = 1.0 / (THETA ** (np.arange(half, dtype=np.float32) / half))
    ang = np.arange(T, dtype=np.float32)[:, None] * inv_freq[None, :]
    # [T, 32] -> [128 partition, NSB, 4 blocks, 3 heads, 32] -> flatten b,h
    def tab12(f):
        t = f(ang).astype(np.float32).reshape(NSB, 4, 128, half)
        t = np.transpose(t, (2, 0, 1, 3))          # [128, NSB, 4, 32]
        t = np.repeat(t[:, :, :, None, :], 3, axis=3)  # [128, NSB, 4, 3, 32]
        return np.ascontiguousarray(t.reshape(128, NSB, 12, 32))
    cos12 = tab12(np.cos)
    sin12 = tab12(np.sin)

    kl = np.arange(128)[None, :, None]
    ql = np.arange(512)[None, None, :]
    iv = np.arange(4)[:, None, None]
    mask = (ql >= kl + 128 * iv).astype(ml_dtypes.bfloat16)
    ident = np.eye(128, dtype=ml_dtypes.bfloat16)
    ones64 = np.ones((1, 64), dtype=ml_dtypes.bfloat16)

    maps = []
    for c in range(NCORES):
        g = c // 2
        wq = np.ascontiguousarray(np.concatenate([
            w_qkv[:, 128 * c:128 * c + 128],          # 2 q heads
            w_qkv[:, 1024 + 64 * g:1024 + 64 * g + 64],   # k group
            w_qkv[:, 1280 + 64 * g:1280 + 64 * g + 64],   # v group
        ], axis=1))
        wo_c = np.ascontiguousarray(w_o[128 * c:128 * c + 128, :])
        maps.append(dict(xb=xb, wqkv=wq, wo=wo_c, cos12=cos12, sin12=sin12,
                         mask=mask, ident=ident, ones64=ones64))
    return maps


def kernel(x, w_qkv, w_o):
    nc = _build()
    maps = host_inputs(x, w_qkv, w_o)
    res = run_bass_kernel_spmd(nc, maps, list(range(NCORES))).results
    y = np.zeros((T, D), np.float64)
    for c in range(NCORES):
        y += np.asarray(res[c]["y"], np.float64)
    return y.astype(np.float32).reshape(1, T, D)



# revision 10
# speedup vs baseline: 2.4078x; 2.4078x over previous
"""Causal GQA attention (qk-norm + rope) on 8 TRN2 NeuronCores.

Sharding: tensor-parallel over heads. Core c owns Q heads {2c, 2c+1} and
KV group c//2 (w_qkv column-parallel, w_o row-parallel). Each core
computes a full-shape partial of the output projection in bf16; the host
sums the 8 partials (row-parallel w_o => partial sums, no on-device
collective).

Per-core pipeline (all matmuls bf16 on PE, fp32 PSUM accumulate), one
For_i timing loop wraps the whole body:
  1. x^T prefetched straight from DRAM via DMA xbar transpose (bf16),
     all 64 block loads issued up front.
  2. qkv = x @ w_qkv_c per 512-row superblock; L2 qk-norm + rope read
     the PSUM accumulator directly. K is NOT normalized on the q/k path:
     1/(8*||k||) is folded into the exp's per-partition scale in phase 2,
     so only the 2 q heads need the normalize multiply.
  3. Flash-style causal attention, both heads fused per k-block
     ([128, 2, 512] tiles): S^T on PE, exp on ACT with per-partition
     kinv scale (scores bounded so no max subtraction), causal mask as
     0/1 bf16 multiply on DVE, A^T V accumulation on PE with an appended
     ones column producing the softmax denominator for free.
  4. y_partial = out_heads @ w_o_rows in bf16, DMA'd out per 128-row tile.
"""

import os

import numpy as np
import ml_dtypes

import concourse.bass as bass
import concourse.tile as tile
from concourse import bacc, mybir
from concourse.bass_utils import run_bass_kernel_spmd

F32 = mybir.dt.float32
BF16 = mybir.dt.bfloat16
AF = mybir.ActivationFunctionType
OP = mybir.AluOpType

T = 4096          # sequence length
D = 1024          # d_model
HD = 64           # head dim
NB = T // 128     # 32 seq blocks of 128
NSB = T // 512    # 8 super blocks of 512
NCORES = 8
THETA = 10000.0

_built = {}


class _nullctx:
    def __enter__(self):
        return None

    def __exit__(self, *a):
        return False


def _emit(tc, nc, xb_d, wqkv_d, wo_d, cos_d, sin_d, mask_d, id_d, y_d):
    fuse = os.environ.get("K_FUSE", "0") == "1"
    with (
        tc.tile_pool(name="pers", bufs=1) as pers,
    ):
        # persistent SBUF tensors
        xT = pers.tile([128, 8, T], BF16)       # x^T, d-chunk j on partitions
        QT = pers.tile([64, 2, T], BF16)        # q-hat^T, both heads
        KT = pers.tile([64, T], BF16)           # k^T (roped, UNnormalized)
        VT = pers.tile([128, NB, 65], BF16)     # per k-block [V | 1]
        kinv = pers.tile([128, NB], F32)        # 1/(8*||k||) per k position
        wqkv_b = pers.tile([128, 8, 256], BF16)
        wo_b = pers.tile([128, D], BF16)
        cos_sb = pers.tile([128, NSB, 12, 32], F32)
        sin_sb = pers.tile([128, NSB, 12, 32], F32)
        mask_sb = pers.tile([128, 4, 512], BF16)
        id_sb = pers.tile([128, 128], BF16)

        nc.sync.dma_start(wqkv_b[:], wqkv_d.rearrange("(j p) c -> p j c", p=128))
        nc.sync.dma_start(wo_b[:], wo_d[:])
        nc.sync.dma_start(cos_sb[:], cos_d[:])
        nc.sync.dma_start(sin_sb[:], sin_d[:])
        nc.sync.dma_start(mask_sb[:], mask_d.rearrange("i p q -> p i q"))
        nc.sync.dma_start(id_sb[:], id_d[:])
        nc.vector.memset(VT[:, :, 64], 1.0)

        rep = int(os.environ.get("K_REP", "1"))
        with (tc.For_i(0, rep, 1,
                       hint_engines=(mybir.EngineType.PE,
                                     mybir.EngineType.Activation,
                                     mybir.EngineType.DVE,
                                     mybir.EngineType.SP))
              if rep != 1 else _nullctx()):
            # prefetch all x^T slices via DMA xbar transpose
            for S in range(NSB):
                for j in range(8):
                    nc.sync.dma_start(
                        xT[:, j, S * 512:(S + 1) * 512],
                        xb_d[S * 512:(S + 1) * 512, 128 * j:128 * (j + 1)],
                        transpose=True)

            # ---- phase 1: qkv projection + qk-norm + rope per superblock
            with (
                tc.tile_pool(name="p1w", bufs=3) as p1w,
                tc.tile_pool(name="p1ps", bufs=2, space="PSUM") as p1ps,
                tc.tile_pool(name="p1pq", bufs=2, space="PSUM") as p1pq,
                tc.tile_pool(name="p1pk", bufs=2, space="PSUM") as p1pk,
            ):
                def emit_tr(S, qhat, khat):
                    # transpose into [hd, s] layout (deferred one superblock
                    # so these PE ops don't stall behind the norm/rope chain)
                    pq = p1pq.tile([64, 2, 4, 128], BF16, tag="pq")
                    pk = p1pk.tile([64, 4, 128], BF16, tag="pk")
                    for b in range(4):
                        for h in range(2):
                            nc.tensor.transpose(pq[:, h, b, :], qhat[:, b, h, :],
                                                id_sb[:])
                        nc.tensor.transpose(pk[:, b, :], khat[:, b, :], id_sb[:])
                    s0 = S * 512
                    nc.scalar.copy(QT[:, :, s0:s0 + 512],
                                   pq.rearrange("p h b s -> p h (b s)"))
                    nc.scalar.copy(KT[:, s0:s0 + 512],
                                   pk.rearrange("p b s -> p (b s)"))

                prev_tr = None
                for S in range(NSB):
                    qkvp = p1ps.tile([128, 4, 256], F32, tag="qkvp")
                    for b in range(4):
                        sb = 4 * S + b
                        for j in range(8):
                            nc.tensor.matmul(qkvp[:, b, :],
                                             xT[:, j, sb * 128:(sb + 1) * 128],
                                             wqkv_b[:, j, :],
                                             start=(j == 0), stop=(j == 7))
                    if prev_tr is not None:
                        emit_tr(*prev_tr)

                    # v slice straight to VT (no norm/rope)
                    nc.vector.tensor_copy(VT[:, 4 * S:4 * S + 4, 0:64],
                                          qkvp[:, :, 192:256])
                    # stage q/k in SBUF so qkvp's PSUM bank frees early
                    qk_s = p1w.tile([128, 4, 192], F32, tag="qk_s")
                    nc.scalar.copy(qk_s[:], qkvp[:, :, 0:192])

                    # squared L2 norms over head_dim
                    sq = p1w.tile([128, 4, 192], F32, tag="sq")
                    nc.scalar.square(sq[:], qk_s[:])
                    ss = p1w.tile([128, 4, 3], F32, tag="ss")
                    nc.vector.reduce_sum(ss[:],
                                         sq.rearrange("p b (h d) -> p b h d", h=3),
                                         axis=mybir.AxisListType.X)
                    srt = p1w.tile([128, 4, 3], F32, tag="srt")
                    nc.scalar.sqrt(srt[:, :, 0:2], ss[:, :, 0:2])
                    # k: sqrt(64*ss) = 8*||k||, folded into exp scale later
                    nc.scalar.activation(srt[:, :, 2:3], ss[:, :, 2:3],
                                         AF.Sqrt, scale=64.0)
                    invq = p1w.tile([128, 4, 2], F32, tag="invq")
                    nc.vector.reciprocal(invq[:], srt[:, :, 0:2])
                    nc.vector.reciprocal(kinv[:, 4 * S:4 * S + 4], srt[:, :, 2])

                    # batched rotate-half rope over [128, 4 blocks, 3 heads, 32]
                    qv = qk_s.rearrange("p b (h d) -> p b h d", h=3)
                    t1, t2 = qv[:, :, :, 0:32], qv[:, :, :, 32:64]
                    cs = cos_sb[:, S].rearrange("p (b h) c -> p b h c", b=4)
                    sn = sin_sb[:, S].rearrange("p (b h) c -> p b h c", b=4)
                    r1 = p1w.tile([128, 4, 3, 32], F32, tag="r1")
                    r2 = p1w.tile([128, 4, 3, 32], F32, tag="r2")
                    rot = p1w.tile([128, 4, 3, 64], F32, tag="rot")
                    nc.vector.tensor_mul(r1[:], t1, cs)
                    nc.vector.tensor_mul(r2[:], t2, sn)
                    nc.vector.tensor_sub(rot[:, :, :, 0:32], r1[:], r2[:])
                    nc.vector.tensor_mul(r1[:], t2, cs)
                    nc.vector.tensor_mul(r2[:], t1, sn)
                    nc.vector.tensor_add(rot[:, :, :, 32:64], r1[:], r2[:])

                    # normalize q heads (cast to bf16); k just casts
                    qhat = p1w.tile([128, 4, 2, 64], BF16, tag="qhat")
                    for b in range(4):
                        for h in range(2):
                            nc.vector.tensor_scalar_mul(
                                qhat[:, b, h], rot[:, b, h],
                                invq[:, b, h:h + 1])
                    khat = p1w.tile([128, 4, 64], BF16, tag="khat")
                    nc.vector.tensor_copy(khat[:], rot[:, :, 2, :])
                    prev_tr = (S, qhat, khat)
                emit_tr(*prev_tr)

            # ---- phase 2: causal attention (heads fused) + output projection
            with (
                tc.tile_pool(name="p2s", bufs=2, space="PSUM") as p2s,
                tc.tile_pool(name="p2av", bufs=1, space="PSUM") as p2av,
                tc.tile_pool(name="p2yp", bufs=2, space="PSUM") as p2yp,
                tc.tile_pool(name="p2sb", bufs=3) as p2sb,
                tc.tile_pool(name="p2n", bufs=2) as p2n,
                tc.tile_pool(name="p2o", bufs=2) as p2o,
                tc.tile_pool(name="p2y", bufs=2) as p2y,
            ):
                def out_proj(qc, ot):
                    # output projection for q chunk qc (deferred one chunk so
                    # these PE ops never head-of-line block the next S/AV
                    # stream while the normalize chain completes)
                    q0 = qc * 512
                    for qb in range(4):
                        ot_blk = ot[:, qb * 128:(qb + 1) * 128]
                        ys = p2y.tile([128, 2, 512], BF16, tag="ys")
                        for nh in range(2):
                            yp = p2yp.tile([128, 512], F32, tag="yp")
                            nc.tensor.matmul(yp[:], ot_blk,
                                             wo_b[:, nh * 512:(nh + 1) * 512],
                                             start=True, stop=True)
                            nc.vector.tensor_copy(ys[:, nh, :], yp[:])
                        nc.sync.dma_start(
                            y_d[q0 + qb * 128:q0 + (qb + 1) * 128, :],
                            ys.rearrange("p a b -> p (a b)"))

                def emit_S_exp(qc, kb):
                    # S^T block + exp (+ causal mask on diagonal blocks)
                    q0 = qc * 512
                    sp = p2s.tile([128, 2, 512], F32, tag="sp")
                    if fuse:
                        nc.tensor.matmul(sp[:, :, :],
                                         KT[:, kb * 128:(kb + 1) * 128],
                                         QT[:, :, q0:q0 + 512],
                                         start=True, stop=True)
                    else:
                        for h in range(2):
                            nc.tensor.matmul(sp[:, h, :],
                                             KT[:, kb * 128:(kb + 1) * 128],
                                             QT[:, h, q0:q0 + 512],
                                             start=True, stop=True)
                    ap = p2sb.tile([128, 2, 512], BF16, tag="ap")
                    nc.scalar.activation(ap[:], sp[:], AF.Exp,
                                         scale=kinv[:, kb:kb + 1])
                    if kb >= 4 * qc:     # diagonal window block
                        i = kb - 4 * qc
                        wm = 128 * (i + 1)   # all-ones past col wm
                        nc.vector.tensor_mul(ap[:, 0, 0:wm], ap[:, 0, 0:wm],
                                             mask_sb[:, i, 0:wm])
                        nc.vector.tensor_mul(ap[:, 1, 0:wm], ap[:, 1, 0:wm],
                                             mask_sb[:, i, 0:wm])
                    return ap

                def emit_AV(av, kb, ap, nkb):
                    if fuse:
                        nc.tensor.matmul(av[:], VT[:, kb, :], ap[:],
                                         start=(kb == 0), stop=(kb == nkb - 1),
                                         skip_group_check=True)
                    else:
                        for h in range(2):
                            nc.tensor.matmul(av[:, h, :], VT[:, kb, :],
                                             ap[:, h, :],
                                             start=(kb == 0),
                                             stop=(kb == nkb - 1),
                                             skip_group_check=True)

                prev = None
                for qc in range(8):          # 512-wide q chunks
                    nkb = 4 * qc + 4
                    av = p2av.tile([65, 2, 512], F32, tag="av")
                    # depth-2 software pipeline: S(kb) runs two blocks ahead
                    # of AV(kb) so the PE stays ahead of ACT's exp stream;
                    # the previous chunk's output projection slots into the
                    # av-buffer turnaround at the chunk boundary
                    pend = []
                    for kb in range(nkb):
                        pend.append(emit_S_exp(qc, kb))
                        if kb == 1 and prev is not None:
                            out_proj(qc - 1, prev)
                        if kb >= 2:
                            emit_AV(av, kb - 2, pend[kb - 2], nkb)
                    emit_AV(av, nkb - 2, pend[nkb - 2], nkb)
                    emit_AV(av, nkb - 1, pend[nkb - 1], nkb)
                    # normalize: row 64 of av is the softmax denominator
                    rec = p2n.tile([1, 2, 512], F32, tag="rec")
                    nc.vector.reciprocal(rec[:], av[64:65, :, :])
                    bcs = p2n.tile([64, 2, 512], F32, tag="bcs")
                    nc.gpsimd.partition_broadcast(bcs[:], rec[:])
                    ot = p2o.tile([128, 512], BF16, tag="ot")
                    nc.vector.tensor_mul(ot[0:64, :], av[0:64, 0, :], bcs[:, 0, :])
                    nc.vector.tensor_mul(ot[64:128, :], av[0:64, 1, :],
                                         bcs[:, 1, :])
                    prev = ot
                out_proj(7, prev)


def _build():
    key = (os.environ.get("K_REP", "1"), os.environ.get("K_FUSE", "0"))
    if key in _built:
        return _built[key]
    nc = bacc.Bacc("TRN2", target_bir_lowering=False, debug=False)
    xb_d = nc.dram_tensor("xb", [T, D], BF16, kind="ExternalInput").ap()
    wqkv_d = nc.dram_tensor("wqkv", [D, 256], BF16, kind="ExternalInput").ap()
    wo_d = nc.dram_tensor("wo", [128, D], BF16, kind="ExternalInput").ap()
    cos_d = nc.dram_tensor("cos12", [128, NSB, 12, 32], F32,
                           kind="ExternalInput").ap()
    sin_d = nc.dram_tensor("sin12", [128, NSB, 12, 32], F32,
                           kind="ExternalInput").ap()
    mask_d = nc.dram_tensor("mask", [4, 128, 512], BF16, kind="ExternalInput").ap()
    id_d = nc.dram_tensor("ident", [128, 128], BF16, kind="ExternalInput").ap()
    y_d = nc.dram_tensor("y", [T, D], BF16, kind="ExternalOutput").ap()
    with tile.TileContext(nc) as tc:
        _emit(tc, nc, xb_d, wqkv_d, wo_d, cos_d, sin_d, mask_d, id_d, y_d)
    nc.compile()
    _built[key] = nc
    return nc


def host_inputs(x, w_qkv, w_o):
    """Per-core input dicts (shards + constant tables)."""
    x2 = np.ascontiguousarray(np.asarray(x, np.float32).reshape(T, D))
    xb = x2.astype(ml_dtypes.bfloat16)
    w_qkv = np.asarray(w_qkv, np.float32)
    w_o = np.asarray(w_o, np.float32)

    half = HD // 2
    inv_freq = 1.0 / (THETA ** (np.arange(half, dtype=np.float32) / half))
    ang = np.arange(T, dtype=np.float32)[:, None] * inv_freq[None, :]
    # [T, 32] -> [128 partition, NSB, 4 blocks, 3 heads, 32] -> flatten b,h
    def tab12(f):
        t = f(ang).astype(np.float32).reshape(NSB, 4, 128, half)
        t = np.transpose(t, (2, 0, 1, 3))          # [128, NSB, 4, 32]
        t = np.repeat(t[:, :, :, None, :], 3, axis=3)  # [128, NSB, 4, 3, 32]
        return np.ascontiguousarray(t.reshape(128, NSB, 12, 32))
    cos12 = tab12(np.cos)
    sin12 = tab12(np.sin)

    kl = np.arange(128)[None, :, None]
    ql = np.arange(512)[None, None, :]
    iv = np.arange(4)[:, None, None]
    mask = (ql >= kl + 128 * iv).astype(ml_dtypes.bfloat16)
    ident = np.eye(128, dtype=ml_dtypes.bfloat16)

    maps = []
    for c in range(NCORES):
        g = c // 2
        wq = np.ascontiguousarray(np.concatenate([
            w_qkv[:, 128 * c:128 * c + 128],          # 2 q heads
            w_qkv[:, 1024 + 64 * g:1024 + 64 * g + 64],   # k group
            w_qkv[:, 1280 + 64 * g:1280 + 64 * g + 64],   # v group
        ], axis=1)).astype(ml_dtypes.bfloat16)
        wo_c = np.ascontiguousarray(
            w_o[128 * c:128 * c + 128, :]).astype(ml_dtypes.bfloat16)
        maps.append(dict(xb=xb, wqkv=wq, wo=wo_c, cos12=cos12, sin12=sin12,
                         mask=mask, ident=ident))
    return maps


def kernel(x, w_qkv, w_o):
    nc = _build()
    maps = host_inputs(x, w_qkv, w_o)
    res = run_bass_kernel_spmd(nc, maps, list(range(NCORES))).results
    y = np.zeros((T, D), np.float64)
    for c in range(NCORES):
        y += np.asarray(res[c]["y"], np.float64)
    return y.astype(np.float32).reshape(1, T, D)


# revision 16
# speedup vs baseline: 2.6155x; 1.0863x over previous
"""Causal GQA attention (qk-norm + rope) on 8 TRN2 NeuronCores.

Sharding: tensor-parallel over heads. Core c owns Q heads {2c, 2c+1} and
KV group c//2 (w_qkv column-parallel, w_o row-parallel). Each core
computes a full-shape partial of the output projection in bf16; the host
sums the 8 partials (row-parallel w_o => partial sums, no on-device
collective).

Per-core pipeline (all matmuls bf16 on PE, fp32 PSUM accumulate), one
For_i timing loop wraps the whole body:
  1. x^T prefetched straight from DRAM via DMA xbar transpose (bf16),
     all 64 block loads issued up front.
  2. qkv = x @ w_qkv_c per 512-row superblock; L2 qk-norm + rope read
     the PSUM accumulator directly. K is NOT normalized on the q/k path:
     1/(8*||k||) is folded into the exp's per-partition scale in phase 2,
     so only the 2 q heads need the normalize multiply.
  3. Flash-style causal attention, both heads fused per k-block
     ([128, 2, 512] tiles): S^T on PE, exp on ACT with per-partition
     kinv scale (scores bounded so no max subtraction), causal mask as
     0/1 bf16 multiply on DVE, A^T V accumulation on PE with an appended
     ones column producing the softmax denominator for free.
  4. y_partial = out_heads @ w_o_rows in bf16, DMA'd out per 128-row tile.
"""

import os

import numpy as np
import ml_dtypes

import concourse.bass as bass
import concourse.tile as tile
from concourse import bacc, mybir
from concourse.bass_utils import run_bass_kernel_spmd

F32 = mybir.dt.float32
BF16 = mybir.dt.bfloat16
AF = mybir.ActivationFunctionType
OP = mybir.AluOpType

T = 4096          # sequence length
D = 1024          # d_model
HD = 64           # head dim
NB = T // 128     # 32 seq blocks of 128
NSB = T // 512    # 8 super blocks of 512
NCORES = 8
THETA = 10000.0

_built = {}


class _nullctx:
    def __enter__(self):
        return None

    def __exit__(self, *a):
        return False


def _emit(tc, nc, xb_d, wqkv_d, wo_d, cos_d, sin_d, mask_d, id_d, y_d):
    fuse = os.environ.get("K_FUSE", "0") == "1"
    with (
        tc.tile_pool(name="pers", bufs=1) as pers,
    ):
        # persistent SBUF tensors
        xT = pers.tile([128, 8, T], BF16)       # x^T, d-chunk j on partitions
        QT = pers.tile([64, 2, T], BF16)        # q-hat^T, both heads
        KT = pers.tile([64, T], BF16)           # k^T (roped, UNnormalized)
        VT = pers.tile([128, NB, 65], BF16)     # per k-block [V | 1]
        kinv = pers.tile([128, NB], F32)        # 1/(8*||k||) per k position
        wqkv_b = pers.tile([128, 8, 256], BF16)
        wo_b = pers.tile([128, D], BF16)
        cos_sb = pers.tile([128, NSB, 12, 32], BF16)
        sin_sb = pers.tile([128, NSB, 12, 32], BF16)
        mask_sb = pers.tile([128, 4, 512], BF16)
        id_sb = pers.tile([128, 128], BF16)

        nc.sync.dma_start(wqkv_b[:], wqkv_d.rearrange("(j p) c -> p j c", p=128))
        nc.sync.dma_start(wo_b[:], wo_d[:])
        nc.sync.dma_start(cos_sb[:], cos_d[:])
        nc.sync.dma_start(sin_sb[:], sin_d[:])
        nc.sync.dma_start(mask_sb[:], mask_d.rearrange("i p q -> p i q"))
        nc.sync.dma_start(id_sb[:], id_d[:])
        nc.vector.memset(VT[:, :, 64], 1.0)

        rep = int(os.environ.get("K_REP", "1"))
        with (tc.For_i(0, rep, 1,
                       hint_engines=(mybir.EngineType.PE,
                                     mybir.EngineType.Activation,
                                     mybir.EngineType.DVE,
                                     mybir.EngineType.SP))
              if rep != 1 else _nullctx()):
            # prefetch all x^T slices via DMA xbar transpose
            for S in range(NSB):
                for j in range(8):
                    nc.sync.dma_start(
                        xT[:, j, S * 512:(S + 1) * 512],
                        xb_d[S * 512:(S + 1) * 512, 128 * j:128 * (j + 1)],
                        transpose=True)

            # ---- phase 1: qkv projection + qk-norm + rope per superblock
            with (
                tc.tile_pool(name="p1w", bufs=4) as p1w,
                tc.tile_pool(name="p1ps", bufs=2, space="PSUM") as p1ps,
                tc.tile_pool(name="p1pq", bufs=2, space="PSUM") as p1pq,
                tc.tile_pool(name="p1pk", bufs=2, space="PSUM") as p1pk,
            ):
                def emit_tr(S, qhat, khat):
                    # transpose into [hd, s] layout (deferred one superblock
                    # so these PE ops don't stall behind the norm/rope chain)
                    pq = p1pq.tile([64, 2, 4, 128], BF16, tag="pq")
                    pk = p1pk.tile([64, 4, 128], BF16, tag="pk")
                    for b in range(4):
                        for h in range(2):
                            nc.tensor.transpose(pq[:, h, b, :], qhat[:, b, h, :],
                                                id_sb[:])
                        nc.tensor.transpose(pk[:, b, :], khat[:, b, :], id_sb[:])
                    s0 = S * 512
                    nc.scalar.copy(QT[:, :, s0:s0 + 512],
                                   pq.rearrange("p h b s -> p h (b s)"))
                    nc.scalar.copy(KT[:, s0:s0 + 512],
                                   pk.rearrange("p b s -> p (b s)"))

                prev_tr = None
                for S in range(NSB):
                    qkvp = p1ps.tile([128, 4, 256], F32, tag="qkvp")
                    for b in range(4):
                        sb = 4 * S + b
                        for j in range(8):
                            nc.tensor.matmul(qkvp[:, b, :],
                                             xT[:, j, sb * 128:(sb + 1) * 128],
                                             wqkv_b[:, j, :],
                                             start=(j == 0), stop=(j == 7))
                    if prev_tr is not None:
                        emit_tr(*prev_tr)

                    # v slice straight to VT (no norm/rope)
                    nc.vector.tensor_copy(VT[:, 4 * S:4 * S + 4, 0:64],
                                          qkvp[:, :, 192:256])
                    # stage q/k in SBUF so qkvp's PSUM bank frees early
                    qk_s = p1w.tile([128, 4, 192], BF16, tag="qk_s")
                    nc.scalar.copy(qk_s[:], qkvp[:, :, 0:192])

                    # squared L2 norms over head_dim
                    sq = p1w.tile([128, 4, 192], BF16, tag="sq")
                    nc.scalar.square(sq[:], qk_s[:])
                    ss = p1w.tile([128, 4, 3], F32, tag="ss")
                    nc.vector.reduce_sum(ss[:],
                                         sq.rearrange("p b (h d) -> p b h d", h=3),
                                         axis=mybir.AxisListType.X)
                    srt = p1w.tile([128, 4, 3], F32, tag="srt")
                    nc.scalar.sqrt(srt[:, :, 0:2], ss[:, :, 0:2])
                    # k: sqrt(64*ss) = 8*||k||, folded into exp scale later
                    nc.scalar.activation(srt[:, :, 2:3], ss[:, :, 2:3],
                                         AF.Sqrt, scale=64.0)
                    invq = p1w.tile([128, 4, 2], F32, tag="invq")
                    nc.vector.reciprocal(invq[:], srt[:, :, 0:2])
                    nc.vector.reciprocal(kinv[:, 4 * S:4 * S + 4], srt[:, :, 2])

                    # batched rotate-half rope over [128, 4 blocks, 3 heads, 32]
                    qv = qk_s.rearrange("p b (h d) -> p b h d", h=3)
                    t1, t2 = qv[:, :, :, 0:32], qv[:, :, :, 32:64]
                    cs = cos_sb[:, S].rearrange("p (b h) c -> p b h c", b=4)
                    sn = sin_sb[:, S].rearrange("p (b h) c -> p b h c", b=4)
                    r1 = p1w.tile([128, 4, 3, 32], BF16, tag="r1")
                    r2 = p1w.tile([128, 4, 3, 32], BF16, tag="r2")
                    rot = p1w.tile([128, 4, 3, 64], BF16, tag="rot")
                    nc.vector.tensor_mul(r1[:], t1, cs)
                    nc.vector.tensor_mul(r2[:], t2, sn)
                    nc.vector.tensor_sub(rot[:, :, :, 0:32], r1[:], r2[:])
                    nc.vector.tensor_mul(r1[:], t2, cs)
                    nc.vector.tensor_mul(r2[:], t1, sn)
                    nc.vector.tensor_add(rot[:, :, :, 32:64], r1[:], r2[:])

                    # normalize q heads (cast to bf16); k just casts
                    qhat = p1w.tile([128, 4, 2, 64], BF16, tag="qhat")
                    for b in range(4):
                        for h in range(2):
                            nc.vector.tensor_scalar_mul(
                                qhat[:, b, h], rot[:, b, h],
                                invq[:, b, h:h + 1])
                    khat = p1w.tile([128, 4, 64], BF16, tag="khat")
                    nc.vector.tensor_copy(khat[:], rot[:, :, 2, :])
                    prev_tr = (S, qhat, khat)
                emit_tr(*prev_tr)

            # ---- phase 2: causal attention (heads fused) + output projection
            with (
                tc.tile_pool(name="p2s", bufs=2, space="PSUM") as p2s,
                tc.tile_pool(name="p2av", bufs=1, space="PSUM") as p2av,
                tc.tile_pool(name="p2yp", bufs=2, space="PSUM") as p2yp,
                tc.tile_pool(name="p2sb", bufs=6) as p2sb,
                tc.tile_pool(name="p2n", bufs=2) as p2n,
                tc.tile_pool(name="p2o", bufs=2) as p2o,
                tc.tile_pool(name="p2y", bufs=2) as p2y,
            ):
                def out_proj(qc, ot):
                    # output projection for q chunk qc (deferred one chunk so
                    # these PE ops never head-of-line block the next S/AV
                    # stream while the normalize chain completes)
                    q0 = qc * 512
                    for qb in range(4):
                        ot_blk = ot[:, qb * 128:(qb + 1) * 128]
                        ys = p2y.tile([128, 2, 512], BF16, tag="ys")
                        for nh in range(2):
                            yp = p2yp.tile([128, 512], F32, tag="yp")
                            nc.tensor.matmul(yp[:], ot_blk,
                                             wo_b[:, nh * 512:(nh + 1) * 512],
                                             start=True, stop=True)
                            nc.vector.tensor_copy(ys[:, nh, :], yp[:])
                        nc.sync.dma_start(
                            y_d[q0 + qb * 128:q0 + (qb + 1) * 128, :],
                            ys.rearrange("p a b -> p (a b)"))

                def emit_S_exp(qc, kb):
                    # S^T block + exp (+ causal mask on diagonal blocks)
                    q0 = qc * 512
                    sp = p2s.tile([128, 2, 512], F32, tag="sp")
                    if fuse:
                        nc.tensor.matmul(sp[:, :, :],
                                         KT[:, kb * 128:(kb + 1) * 128],
                                         QT[:, :, q0:q0 + 512],
                                         start=True, stop=True)
                    else:
                        for h in range(2):
                            nc.tensor.matmul(sp[:, h, :],
                                             KT[:, kb * 128:(kb + 1) * 128],
                                             QT[:, h, q0:q0 + 512],
                                             start=True, stop=True)
                    ap = p2sb.tile([128, 2, 512], BF16, tag="ap")
                    c0 = 0
                    if kb >= 4 * qc:     # diagonal window block
                        c0 = 128 * (kb - 4 * qc)   # cols < c0 fully masked
                    if c0 == 0:
                        nc.scalar.activation(ap[:], sp[:], AF.Exp,
                                             scale=kinv[:, kb:kb + 1])
                    else:
                        # per-head 1D-free slices: a 2D-free AP with column
                        # offset lowers incorrectly on HW (CoreSim-only pass)
                        for h in range(2):
                            nc.scalar.activation(ap[:, h, c0:512],
                                                 sp[:, h, c0:512], AF.Exp,
                                                 scale=kinv[:, kb:kb + 1])
                    if kb >= 4 * qc:
                        i = kb - 4 * qc
                        nc.vector.tensor_mul(ap[:, 0, c0:c0 + 128],
                                             ap[:, 0, c0:c0 + 128],
                                             mask_sb[:, i, c0:c0 + 128])
                        nc.vector.tensor_mul(ap[:, 1, c0:c0 + 128],
                                             ap[:, 1, c0:c0 + 128],
                                             mask_sb[:, i, c0:c0 + 128])
                    return ap, c0

                def emit_AV(av, kb, ap_c0, nkb):
                    ap, c0 = ap_c0
                    for h in range(2):
                        nc.tensor.matmul(av[:, h, c0:512], VT[:, kb, :],
                                         ap[:, h, c0:512],
                                         start=(kb == 0),
                                         stop=(kb == nkb - 1),
                                         skip_group_check=True)

                prev = None
                for qc in range(8):          # 512-wide q chunks
                    nkb = 4 * qc + 4
                    av = p2av.tile([65, 2, 512], F32, tag="av")
                    # depth-2 software pipeline: S(kb) runs two blocks ahead
                    # of AV(kb) so the PE stays ahead of ACT's exp stream;
                    # the previous chunk's output projection slots into the
                    # av-buffer turnaround at the chunk boundary
                    pend = []
                    for kb in range(nkb):
                        pend.append(emit_S_exp(qc, kb))
                        if kb == 1 and prev is not None:
                            out_proj(qc - 1, prev)
                        if kb >= 4:
                            emit_AV(av, kb - 4, pend[kb - 4], nkb)
                    for k in range(max(0, nkb - 4), nkb):
                        emit_AV(av, k, pend[k], nkb)
                    # normalize: row 64 of av is the softmax denominator
                    rec = p2n.tile([1, 2, 512], F32, tag="rec")
                    nc.vector.reciprocal(rec[:], av[64:65, :, :])
                    bcs = p2n.tile([64, 2, 512], F32, tag="bcs")
                    nc.gpsimd.partition_broadcast(bcs[:], rec[:])
                    ot = p2o.tile([128, 512], BF16, tag="ot")
                    nc.vector.tensor_mul(ot[0:64, :], av[0:64, 0, :], bcs[:, 0, :])
                    nc.vector.tensor_mul(ot[64:128, :], av[0:64, 1, :],
                                         bcs[:, 1, :])
                    prev = ot
                out_proj(7, prev)


def _build():
    key = (os.environ.get("K_REP", "1"), os.environ.get("K_FUSE", "0"))
    if key in _built:
        return _built[key]
    nc = bacc.Bacc("TRN2", target_bir_lowering=False, debug=False)
    xb_d = nc.dram_tensor("xb", [T, D], BF16, kind="ExternalInput").ap()
    wqkv_d = nc.dram_tensor("wqkv", [D, 256], BF16, kind="ExternalInput").ap()
    wo_d = nc.dram_tensor("wo", [128, D], BF16, kind="ExternalInput").ap()
    cos_d = nc.dram_tensor("cos12", [128, NSB, 12, 32], BF16,
                           kind="ExternalInput").ap()
    sin_d = nc.dram_tensor("sin12", [128, NSB, 12, 32], BF16,
                           kind="ExternalInput").ap()
    mask_d = nc.dram_tensor("mask", [4, 128, 512], BF16, kind="ExternalInput").ap()
    id_d = nc.dram_tensor("ident", [128, 128], BF16, kind="ExternalInput").ap()
    y_d = nc.dram_tensor("y", [T, D], BF16, kind="ExternalOutput").ap()
    with tile.TileContext(nc) as tc:
        _emit(tc, nc, xb_d, wqkv_d, wo_d, cos_d, sin_d, mask_d, id_d, y_d)
    nc.compile()
    _built[key] = nc
    return nc


def host_inputs(x, w_qkv, w_o):
    """Per-core input dicts (shards + constant tables)."""
    x2 = np.ascontiguousarray(np.asarray(x, np.float32).reshape(T, D))
    xb = x2.astype(ml_dtypes.bfloat16)
    w_qkv = np.asarray(w_qkv, np.float32)
    w_o = np.asarray(w_o, np.float32)

    half = HD // 2
    inv_freq = 1.0 / (THETA ** (np.arange(half, dtype=np.float32) / half))
    ang = np.arange(T, dtype=np.float32)[:, None] * inv_freq[None, :]
    # [T, 32] -> [128 partition, NSB, 4 blocks, 3 heads, 32] -> flatten b,h
    def tab12(f):
        t = f(ang).astype(np.float32).reshape(NSB, 4, 128, half)
        t = np.transpose(t, (2, 0, 1, 3))          # [128, NSB, 4, 32]
        t = np.repeat(t[:, :, :, None, :], 3, axis=3)  # [128, NSB, 4, 3, 32]
        return np.ascontiguousarray(
            t.reshape(128, NSB, 12, 32)).astype(ml_dtypes.bfloat16)
    cos12 = tab12(np.cos)
    sin12 = tab12(np.sin)

    kl = np.arange(128)[None, :, None]
    ql = np.arange(512)[None, None, :]
    iv = np.arange(4)[:, None, None]
    mask = (ql >= kl + 128 * iv).astype(ml_dtypes.bfloat16)
    ident = np.eye(128, dtype=ml_dtypes.bfloat16)

    maps = []
    for c in range(NCORES):
        g = c // 2
        wq = np.ascontiguousarray(np.concatenate([
            w_qkv[:, 128 * c:128 * c + 128],          # 2 q heads
            w_qkv[:, 1024 + 64 * g:1024 + 64 * g + 64],   # k group
            w_qkv[:, 1280 + 64 * g:1280 + 64 * g + 64],   # v group
        ], axis=1)).astype(ml_dtypes.bfloat16)
        wo_c = np.ascontiguousarray(
            w_o[128 * c:128 * c + 128, :]).astype(ml_dtypes.bfloat16)
        maps.append(dict(xb=xb, wqkv=wq, wo=wo_c, cos12=cos12, sin12=sin12,
                         mask=mask, ident=ident))
    return maps


def kernel(x, w_qkv, w_o):
    nc = _build()
    maps = host_inputs(x, w_qkv, w_o)
    res = run_bass_kernel_spmd(nc, maps, list(range(NCORES))).results
    y = np.zeros((T, D), np.float64)
    for c in range(NCORES):
        y += np.asarray(res[c]["y"], np.float64)
    return y.astype(np.float32).reshape(1, T, D)


# revision 18
# speedup vs baseline: 2.8796x; 1.1010x over previous
"""Causal GQA attention (qk-norm + rope) on 8 TRN2 NeuronCores.

Sharding: tensor-parallel over heads. Core c owns Q heads {2c, 2c+1} and
KV group c//2 (w_qkv column-parallel, w_o row-parallel). Each core
computes a full-shape partial of the output projection in bf16; the host
sums the 8 partials (row-parallel w_o => partial sums, no on-device
collective).

Per-core pipeline (all matmuls bf16 on PE, fp32 PSUM accumulate), one
For_i timing loop wraps the whole body:
  1. x^T prefetched straight from DRAM via DMA xbar transpose (bf16),
     all 64 block loads issued up front.
  2. qkv = x @ w_qkv_c per 512-row superblock; L2 qk-norm + rope read
     the PSUM accumulator directly. K is NOT normalized on the q/k path:
     1/(8*||k||) is folded into the exp's per-partition scale in phase 2,
     so only the 2 q heads need the normalize multiply.
  3. Flash-style causal attention, both heads fused per k-block
     ([128, 2, 512] tiles): S^T on PE, exp on ACT with per-partition
     kinv scale (scores bounded so no max subtraction), causal mask as
     0/1 bf16 multiply on DVE, A^T V accumulation on PE with an appended
     ones column producing the softmax denominator for free.
  4. y_partial = out_heads @ w_o_rows in bf16, DMA'd out per 128-row tile.
"""

import os

import numpy as np
import ml_dtypes

import concourse.bass as bass
import concourse.tile as tile
from concourse import bacc, mybir
from concourse.bass_utils import run_bass_kernel_spmd

F32 = mybir.dt.float32
BF16 = mybir.dt.bfloat16
AF = mybir.ActivationFunctionType
OP = mybir.AluOpType

T = 4096          # sequence length
D = 1024          # d_model
HD = 64           # head dim
NB = T // 128     # 32 seq blocks of 128
NSB = T // 512    # 8 super blocks of 512
NCORES = 8
THETA = 10000.0

_built = {}


class _nullctx:
    def __enter__(self):
        return None

    def __exit__(self, *a):
        return False


def _emit(tc, nc, xb_d, wqkv_d, wo_d, cos_d, sin_d, mask_d, id_d, y_d):
    fuse = os.environ.get("K_FUSE", "0") == "1"
    with (
        tc.tile_pool(name="pers", bufs=1) as pers,
    ):
        # persistent SBUF tensors
        xT = pers.tile([128, 8, T], BF16)       # x^T, d-chunk j on partitions
        QT = pers.tile([64, 2, T], BF16)        # q-hat^T, both heads
        KT = pers.tile([64, T], BF16)           # k^T (roped, UNnormalized)
        VT = pers.tile([128, NB, 65], BF16)     # per k-block [V | 1]
        kinv = pers.tile([128, NB], F32)        # 1/(8*||k||) per k position
        wqkv_b = pers.tile([128, 8, 256], BF16)
        wo_b = pers.tile([128, D], BF16)
        cos_sb = pers.tile([128, NSB, 12, 32], BF16)
        sin_sb = pers.tile([128, NSB, 12, 32], BF16)
        mask_sb = pers.tile([128, 4, 512], BF16)
        id_sb = pers.tile([128, 128], BF16)

        nc.sync.dma_start(wqkv_b[:], wqkv_d.rearrange("(j p) c -> p j c", p=128))
        nc.sync.dma_start(wo_b[:], wo_d[:])
        nc.sync.dma_start(cos_sb[:], cos_d[:])
        nc.sync.dma_start(sin_sb[:], sin_d[:])
        nc.sync.dma_start(mask_sb[:], mask_d.rearrange("i p q -> p i q"))
        nc.sync.dma_start(id_sb[:], id_d[:])
        nc.vector.memset(VT[:, :, 64], 1.0)

        rep = int(os.environ.get("K_REP", "1"))
        with (tc.For_i(0, rep, 1,
                       hint_engines=(mybir.EngineType.PE,
                                     mybir.EngineType.Activation,
                                     mybir.EngineType.DVE,
                                     mybir.EngineType.SP))
              if rep != 1 else _nullctx()):
            # prefetch all x^T slices via DMA xbar transpose
            for S in range(NSB):
                for j in range(8):
                    nc.sync.dma_start(
                        xT[:, j, S * 512:(S + 1) * 512],
                        xb_d[S * 512:(S + 1) * 512, 128 * j:128 * (j + 1)],
                        transpose=True)

            # ---- phase 1: qkv projection + qk-norm + rope per superblock
            with (
                tc.tile_pool(name="p1w", bufs=4) as p1w,
                tc.tile_pool(name="p1ps", bufs=2, space="PSUM") as p1ps,
                tc.tile_pool(name="p1pq", bufs=2, space="PSUM") as p1pq,
                tc.tile_pool(name="p1pk", bufs=2, space="PSUM") as p1pk,
            ):
                def emit_tr(S, qhat, khat):
                    # transpose into [hd, s] layout (deferred one superblock
                    # so these PE ops don't stall behind the norm/rope chain)
                    pq = p1pq.tile([64, 2, 4, 128], BF16, tag="pq")
                    pk = p1pk.tile([64, 4, 128], BF16, tag="pk")
                    for b in range(4):
                        for h in range(2):
                            nc.tensor.transpose(pq[:, h, b, :], qhat[:, b, h, :],
                                                id_sb[:])
                        nc.tensor.transpose(pk[:, b, :], khat[:, b, :], id_sb[:])
                    s0 = S * 512
                    nc.scalar.copy(QT[:, :, s0:s0 + 512],
                                   pq.rearrange("p h b s -> p h (b s)"))
                    nc.scalar.copy(KT[:, s0:s0 + 512],
                                   pk.rearrange("p b s -> p (b s)"))

                prev_tr = None
                for S in range(NSB):
                    qkvp = p1ps.tile([128, 4, 256], F32, tag="qkvp")
                    for b in range(4):
                        sb = 4 * S + b
                        for j in range(8):
                            nc.tensor.matmul(qkvp[:, b, :],
                                             xT[:, j, sb * 128:(sb + 1) * 128],
                                             wqkv_b[:, j, :],
                                             start=(j == 0), stop=(j == 7))
                    if prev_tr is not None:
                        emit_tr(*prev_tr)

                    # v slice straight to VT (no norm/rope)
                    nc.vector.tensor_copy(VT[:, 4 * S:4 * S + 4, 0:64],
                                          qkvp[:, :, 192:256])
                    # stage q/k in SBUF so qkvp's PSUM bank frees early
                    qk_s = p1w.tile([128, 4, 192], BF16, tag="qk_s")
                    nc.scalar.copy(qk_s[:], qkvp[:, :, 0:192])

                    # squared L2 norms over head_dim
                    sq = p1w.tile([128, 4, 192], BF16, tag="sq")
                    nc.scalar.square(sq[:], qk_s[:])
                    ss = p1w.tile([128, 4, 3], F32, tag="ss")
                    nc.vector.reduce_sum(ss[:],
                                         sq.rearrange("p b (h d) -> p b h d", h=3),
                                         axis=mybir.AxisListType.X)
                    srt = p1w.tile([128, 4, 3], F32, tag="srt")
                    nc.scalar.sqrt(srt[:, :, 0:2], ss[:, :, 0:2])
                    # k: sqrt(64*ss) = 8*||k||, folded into exp scale later
                    nc.scalar.activation(srt[:, :, 2:3], ss[:, :, 2:3],
                                         AF.Sqrt, scale=64.0)
                    invq = p1w.tile([128, 4, 2], F32, tag="invq")
                    nc.vector.reciprocal(invq[:], srt[:, :, 0:2])
                    nc.vector.reciprocal(kinv[:, 4 * S:4 * S + 4], srt[:, :, 2])

                    # batched rotate-half rope over [128, 4 blocks, 3 heads, 32]
                    qv = qk_s.rearrange("p b (h d) -> p b h d", h=3)
                    t1, t2 = qv[:, :, :, 0:32], qv[:, :, :, 32:64]
                    cs = cos_sb[:, S].rearrange("p (b h) c -> p b h c", b=4)
                    sn = sin_sb[:, S].rearrange("p (b h) c -> p b h c", b=4)
                    r1 = p1w.tile([128, 4, 3, 32], BF16, tag="r1")
                    r2 = p1w.tile([128, 4, 3, 32], BF16, tag="r2")
                    rot = p1w.tile([128, 4, 3, 64], BF16, tag="rot")
                    nc.vector.tensor_mul(r1[:], t1, cs)
                    nc.vector.tensor_mul(r2[:], t2, sn)
                    nc.vector.tensor_sub(rot[:, :, :, 0:32], r1[:], r2[:])
                    nc.vector.tensor_mul(r1[:], t2, cs)
                    nc.vector.tensor_mul(r2[:], t1, sn)
                    nc.vector.tensor_add(rot[:, :, :, 32:64], r1[:], r2[:])

                    # normalize q heads (cast to bf16); k just casts
                    qhat = p1w.tile([128, 4, 2, 64], BF16, tag="qhat")
                    for b in range(4):
                        for h in range(2):
                            nc.vector.tensor_scalar_mul(
                                qhat[:, b, h], rot[:, b, h],
                                invq[:, b, h:h + 1])
                    khat = p1w.tile([128, 4, 64], BF16, tag="khat")
                    nc.vector.tensor_copy(khat[:], rot[:, :, 2, :])
                    prev_tr = (S, qhat, khat)
                emit_tr(*prev_tr)

            # ---- phase 2: causal attention (heads fused) + output projection
            with (
                tc.tile_pool(name="p2s", bufs=2, space="PSUM") as p2s,
                tc.tile_pool(name="p2av", bufs=1, space="PSUM") as p2av,
                tc.tile_pool(name="p2yp", bufs=2, space="PSUM") as p2yp,
                tc.tile_pool(name="p2sb", bufs=4) as p2sb,
                tc.tile_pool(name="p2n", bufs=2) as p2n,
                tc.tile_pool(name="p2o", bufs=2) as p2o,
                tc.tile_pool(name="p2y", bufs=2) as p2y,
            ):
                def out_proj(qc, ot):
                    # output projection for q chunk qc (deferred one chunk so
                    # these PE ops never head-of-line block the next S/AV
                    # stream while the normalize chain completes)
                    q0 = qc * 512
                    for qb in range(4):
                        ot_blk = ot[:, qb * 128:(qb + 1) * 128]
                        ys = p2y.tile([128, 2, 512], BF16, tag="ys")
                        for nh in range(2):
                            yp = p2yp.tile([128, 512], F32, tag="yp")
                            nc.tensor.matmul(yp[:], ot_blk,
                                             wo_b[:, nh * 512:(nh + 1) * 512],
                                             start=True, stop=True)
                            nc.vector.tensor_copy(ys[:, nh, :], yp[:])
                        nc.sync.dma_start(
                            y_d[q0 + qb * 128:q0 + (qb + 1) * 128, :],
                            ys.rearrange("p a b -> p (a b)"))

                def emit_S_exp(qc, kb):
                    # S^T block + exp (+ causal mask on diagonal blocks)
                    q0 = qc * 512
                    sp = p2s.tile([128, 2, 512], F32, tag="sp")
                    if fuse:
                        nc.tensor.matmul(sp[:, :, :],
                                         KT[:, kb * 128:(kb + 1) * 128],
                                         QT[:, :, q0:q0 + 512],
                                         start=True, stop=True)
                    else:
                        for h in range(2):
                            nc.tensor.matmul(sp[:, h, :],
                                             KT[:, kb * 128:(kb + 1) * 128],
                                             QT[:, h, q0:q0 + 512],
                                             start=True, stop=True)
                    ap = p2sb.tile([128, 2, 512], BF16, tag="ap")
                    c0 = 0
                    if kb >= 4 * qc:     # diagonal window block
                        c0 = 128 * (kb - 4 * qc)   # cols < c0 fully masked
                    # exp is full-width (one op; the HW per-op overhead beats
                    # the skipped elements) but AV still skips cols < c0 --
                    # those ap values are valid exps that simply go unread
                    nc.scalar.activation(ap[:], sp[:], AF.Exp,
                                         scale=kinv[:, kb:kb + 1])
                    if kb >= 4 * qc:
                        i = kb - 4 * qc
                        nc.vector.tensor_mul(ap[:, 0, c0:c0 + 128],
                                             ap[:, 0, c0:c0 + 128],
                                             mask_sb[:, i, c0:c0 + 128])
                        nc.vector.tensor_mul(ap[:, 1, c0:c0 + 128],
                                             ap[:, 1, c0:c0 + 128],
                                             mask_sb[:, i, c0:c0 + 128])
                    return ap, c0

                def emit_AV(av, kb, ap_c0, nkb):
                    ap, c0 = ap_c0
                    for h in range(2):
                        nc.tensor.matmul(av[:, h, c0:512], VT[:, kb, :],
                                         ap[:, h, c0:512],
                                         start=(kb == 0),
                                         stop=(kb == nkb - 1),
                                         skip_group_check=True)

                prev = None
                for qc in range(8):          # 512-wide q chunks
                    nkb = 4 * qc + 4
                    av = p2av.tile([65, 2, 512], F32, tag="av")
                    # depth-2 software pipeline: S(kb) runs two blocks ahead
                    # of AV(kb) so the PE stays ahead of ACT's exp stream;
                    # the previous chunk's output projection slots into the
                    # av-buffer turnaround at the chunk boundary
                    pend = []
                    for kb in range(nkb):
                        pend.append(emit_S_exp(qc, kb))
                        if kb == 1 and prev is not None:
                            out_proj(qc - 1, prev)
                        if kb >= 2:
                            emit_AV(av, kb - 2, pend[kb - 2], nkb)
                    for k in range(max(0, nkb - 2), nkb):
                        emit_AV(av, k, pend[k], nkb)
                    # normalize: row 64 of av is the softmax denominator
                    rec = p2n.tile([1, 2, 512], F32, tag="rec")
                    nc.vector.reciprocal(rec[:], av[64:65, :, :])
                    bcs = p2n.tile([64, 2, 512], F32, tag="bcs")
                    nc.gpsimd.partition_broadcast(bcs[:], rec[:])
                    ot = p2o.tile([128, 512], BF16, tag="ot")
                    nc.vector.tensor_mul(ot[0:64, :], av[0:64, 0, :], bcs[:, 0, :])
                    nc.vector.tensor_mul(ot[64:128, :], av[0:64, 1, :],
                                         bcs[:, 1, :])
                    prev = ot
                out_proj(7, prev)


def _build():
    key = (os.environ.get("K_REP", "1"), os.environ.get("K_FUSE", "0"))
    if key in _built:
        return _built[key]
    nc = bacc.Bacc("TRN2", target_bir_lowering=False, debug=False)
    xb_d = nc.dram_tensor("xb", [T, D], BF16, kind="ExternalInput").ap()
    wqkv_d = nc.dram_tensor("wqkv", [D, 256], BF16, kind="ExternalInput").ap()
    wo_d = nc.dram_tensor("wo", [128, D], BF16, kind="ExternalInput").ap()
    cos_d = nc.dram_tensor("cos12", [128, NSB, 12, 32], BF16,
                           kind="ExternalInput").ap()
    sin_d = nc.dram_tensor("sin12", [128, NSB, 12, 32], BF16,
                           kind="ExternalInput").ap()
    mask_d = nc.dram_tensor("mask", [4, 128, 512], BF16, kind="ExternalInput").ap()
    id_d = nc.dram_tensor("ident", [128, 128], BF16, kind="ExternalInput").ap()
    y_d = nc.dram_tensor("y", [T, D], BF16, kind="ExternalOutput").ap()
    with tile.TileContext(nc) as tc:
        _emit(tc, nc, xb_d, wqkv_d, wo_d, cos_d, sin_d, mask_d, id_d, y_d)
    nc.compile()
    _built[key] = nc
    return nc


def host_inputs(x, w_qkv, w_o):
    """Per-core input dicts (shards + constant tables)."""
    x2 = np.ascontiguousarray(np.asarray(x, np.float32).reshape(T, D))
    xb = x2.astype(ml_dtypes.bfloat16)
    w_qkv = np.asarray(w_qkv, np.float32)
    w_o = np.asarray(w_o, np.float32)

    half = HD // 2
    inv_freq = 1.0 / (THETA ** (np.arange(half, dtype=np.float32) / half))
    ang = np.arange(T, dtype=np.float32)[:, None] * inv_freq[None, :]
    # [T, 32] -> [128 partition, NSB, 4 blocks, 3 heads, 32] -> flatten b,h
    def tab12(f):
        t = f(ang).astype(np.float32).reshape(NSB, 4, 128, half)
        t = np.transpose(t, (2, 0, 1, 3))          # [128, NSB, 4, 32]
        t = np.repeat(t[:, :, :, None, :], 3, axis=3)  # [128, NSB, 4, 3, 32]
        return np.ascontiguousarray(
            t.reshape(128, NSB, 12, 32)).astype(ml_dtypes.bfloat16)
    cos12 = tab12(np.cos)
    sin12 = tab12(np.sin)

    kl = np.arange(128)[None, :, None]
    ql = np.arange(512)[None, None, :]
    iv = np.arange(4)[:, None, None]
    mask = (ql >= kl + 128 * iv).astype(ml_dtypes.bfloat16)
    ident = np.eye(128, dtype=ml_dtypes.bfloat16)

    maps = []
    for c in range(NCORES):
        g = c // 2
        wq = np.ascontiguousarray(np.concatenate([
            w_qkv[:, 128 * c:128 * c + 128],          # 2 q heads
            w_qkv[:, 1024 + 64 * g:1024 + 64 * g + 64],   # k group
            w_qkv[:, 1280 + 64 * g:1280 + 64 * g + 64],   # v group
        ], axis=1)).astype(ml_dtypes.bfloat16)
        wo_c = np.ascontiguousarray(
            w_o[128 * c:128 * c + 128, :]).astype(ml_dtypes.bfloat16)
        maps.append(dict(xb=xb, wqkv=wq, wo=wo_c, cos12=cos12, sin12=sin12,
                         mask=mask, ident=ident))
    return maps


def kernel(x, w_qkv, w_o):
    nc = _build()
    maps = host_inputs(x, w_qkv, w_o)
    res = run_bass_kernel_spmd(nc, maps, list(range(NCORES))).results
    y = np.zeros((T, D), np.float64)
    for c in range(NCORES):
        y += np.asarray(res[c]["y"], np.float64)
    return y.astype(np.float32).reshape(1, T, D)
